# revision 48
# baseline (speedup 1.0000x reference)
"""Trainium2 Bass kernel for nn_Block_11166914969721 (dense transformer block).

Sharding: 8 cores = (batch b in {0,1}) x (query chunk qc in {0..3}, 1024
queries each). Each core recomputes the full KV side for its batch and
computes attention + proj + MLP for its own query chunk.

Key implementation points (v2, low-DMA):
- All activations in T-layout [channels(part), tokens(free)].
- Projection weights are host-padded into 64-row head slots (zeros in the
  pad rows) so each 128-row PSUM co-tile evicts with a single vector op
  directly into the persistent q/k/ocat layouts -- no scatter DMAs.
- LayerNorm stats stay on chip: ones-matmul column sums -> PE transpose to
  [tokens(part), 2] -> vector math -> PE transpose back to rows -> K=1
  f32r ones-matmul broadcast to [128, tok] PSUM tiles.
- Depthwise 3x3x3 conv = 27 PSUM-accumulated diag matmuls on shifted views
  of a zero-padded buffer; kv-proj evictions write the padded buffer
  interior directly. Transposed 49-augmented V tiles are written with
  direct PSUM->SBUF copies.
- Attention is software-pipelined: S(u+1) is issued before AV(u) so the
  Act engine's exp stream never starves; softmax denominators come from an
  appended ones-column on V, inverted on chip and broadcast with K=1
  matmuls.
- The MLP for query block 0 is issue-interleaved under attention of query
  block 1.
"""

import os
import numpy as np
import ml_dtypes

import concourse.bass as bass
import concourse.mybir as mybir
import concourse.tile as tile
from concourse.bass_utils import run_bass_kernel_spmd
from concourse.masks import make_identity
from concourse.vector_clock import ScopedClock

BF = ml_dtypes.bfloat16
AL = mybir.AluOpType
AF = mybir.ActivationFunctionType
F32 = mybir.dt.float32
F32R = mybir.dt.float32r
BF16 = mybir.dt.bfloat16

# ---------------------------------------------------------------------------
# Workarounds: walrus in this container accepts at most ONE sem-wait per
# instruction. (a) Tile's kernel-tail drain aggregates one wait per live
# proc -> spread across SP nops. (b) Mid-kernel instructions may also get
# several waits -> post-pass splits them onto same-engine NoOps.
# ---------------------------------------------------------------------------


def _patched_drain_and_barrier(self, tick_clock, wait_clock):
    nc = self.nc
    collector = nc.sync.nop(nofuse=True)
    wait_clock.add_sem_waits(collector.ins, ScopedClock({None: tick_clock.global_clock}))
    si = collector.ins.sync_info
    waits = list(si.on_wait) if si is not None and si.on_wait else []
    if si is not None:
        si.on_wait = waits[:1]
    for i in range(1, len(waits)):
        nop = nc.sync.nop(nofuse=True)
        nop.ins.sync_info = mybir.SyncInfo(on_wait=waits[i:i + 1], on_update=[])
    nc.sync.drain()
    nc.all_engine_barrier()
    assert self.sems is not None
    popped = nc._tile_sem_poison_stack.pop()
    assert popped is self._sem_poison
    nc.clear_and_free_semaphores(list(self.sems.allocated().values()))
    nc.all_engine_barrier()


tile.TileContext._drain_and_barrier = _patched_drain_and_barrier


def _split_multi_waits(nc):
    cnt = 0
    for fn in nc.m.functions:
        for bb in fn.blocks:
            out = []
            for inst in bb.instructions:
                si = inst.sync_info
                if si is not None and si.on_wait and len(si.on_wait) > 1:
                    waits = list(si.on_wait)
                    for w in waits[:-1]:
                        cnt += 1
                        out.append(mybir.InstNoOp(
                            name=f"nwsplit{cnt}",
                            engine=inst.engine,
                            sync_info=mybir.SyncInfo(on_wait=[w], on_update=[]),
                            bass_nofuse=True))
                    si.on_wait = waits[-1:]
                out.append(inst)
            bb.instructions[:] = out
    return cnt


# ---------------------------------------------------------------------------
B, N, C = 2, 4096, 384
HD = 48
C2 = 192
N1 = 512
HID = 4 * C
NQ = 1024          # queries per core
CT = 3             # channel tiles of 128
EPS = 1e-5
ABL = os.environ.get("KABL", "")


def _v49_runs(c0, ln):
    """channel range of v -> 49-augmented column offsets: (src_off, len, dst_col)."""
    out = []
    bs = sorted(set([c0, c0 + ln] + [k * 48 for k in range(1, 4) if c0 < k * 48 < c0 + ln]))
    for a, b in zip(bs, bs[1:]):
        out.append((a - c0, b - a, (a // 48) * 49 + a % 48))
    return out


def build_program():
    nc = bass.Bass()
    d = {}

    def din(name, shape, dt):
        d[name] = nc.dram_tensor(name, shape, dt, kind="ExternalInput")

    din("xt", [C, N], BF16)
    din("xct", [C, NQ], F32)
    din("vecs", [128, 50], F32)     # all bias/affine vectors, pre-packed
    din("sel2", [2, 256], BF16)     # one-hot row selectors (K=2)
    din("sel64", [64, 96], F32)     # one-hot selectors rows 16/48 (K=64)
    din("wcat", [C, 1920], BF16)    # qw(512) | kv2w(512) | kv1w(512) | s2w(384)
    din("s1w", [8, C, C], BF16)
    din("dga", [128, 54, 128], BF16)  # dg2a | dg1a
    din("dgb", [64, 54, 64], BF16)    # dg2b | dg1b
    din("pw", [512, C], BF16)       # padded input rows
    din("f1w", [C, HID], BF16)
    din("f2w", [HID, C], BF16)

    out_d = nc.dram_tensor("out", [NQ, C], F32, kind="ExternalOutput")

    with tile.TileContext(nc, pool_alloc_mode="queue") as tc:
        _body(tc, nc, d, out_d)
    _split_multi_waits(nc)
    return nc


def _body(tc, nc, d, out_d):
    from contextlib import ExitStack

    dma = nc.gpsimd.dma_start

    ctx = ExitStack()
    with ctx:
        glob = ctx.enter_context(tc.tile_pool(name="glob", bufs=1))
        wpool = ctx.enter_context(tc.tile_pool(name="wpool", bufs=1))
        rows = ctx.enter_context(tc.tile_pool(name="rows", bufs=2))

        ones_b = glob.tile([128, 1], BF16, tag="ones_b")
        nc.vector.memset(ones_b, 1.0)
        ones_f = glob.tile([128, 1], F32, tag="ones_f")
        nc.vector.memset(ones_f, 1.0)
        onesrow_f = glob.tile([1, 128], F32, tag="onesrow_f")
        nc.vector.memset(onesrow_f, 1.0)
        # row-selector lhsT matrices (loaded: partition-offset memsets are
        # not legal engine ops): sel2[:, j, :] one-hot row j (K=2);
        # sel64[:, j, :] one-hot row 16/48 (K=64, for denominator rows)
        sel2t = glob.tile([2, 256], BF16, tag="sel2")
        dma(out=sel2t, in_=d["sel2"][:, :])
        sel2 = sel2t.rearrange("p (j c) -> p j c", j=2)
        sel64t = glob.tile([64, 96], F32, tag="sel64")
        dma(out=sel64t, in_=d["sel64"][:, :])
        sel64 = sel64t.rearrange("p (j c) -> p j c", j=2)
        eps_t = glob.tile([128, 1], F32, tag="eps")
        nc.vector.memset(eps_t, EPS)
        ident = glob.tile([128, 128], BF16, tag="ident")
        make_identity(nc, ident)
        identf = glob.tile([128, 128], F32, tag="identf")
        make_identity(nc, identf)

        def mat_sb(name, ktiles, cols, pool, tag=None):
            t = pool.tile([128, ktiles, cols], BF16, tag=tag or f"m_{name}", name=name)
            dma(out=t, in_=d[name].rearrange("(t p) co -> p t co", p=128))
            return t

        vecs = wpool.tile([128, 50], F32, tag="vecs")
        dma(out=vecs, in_=d["vecs"][:, :])
        qb_sb = vecs[:, 0:4]
        s2b_sb = vecs[:, 4:7]
        s1b_sb = vecs[:, 7:10]
        n1w_sb = vecs[:, 10:13]
        n1b_sb = vecs[:, 13:16]
        n2w_sb = vecs[:, 16:19]
        n2b_sb = vecs[:, 19:22]
        pb_sb = vecs[:, 22:25]
        ln2w_sb = vecs[:, 25:28]
        ln2b_sb = vecs[:, 28:31]
        f2b_sb = vecs[:, 31:34]
        f1b_sb = vecs[:, 34:46]
        lcb = {1: (vecs[:, 46:47], vecs[0:64, 47:48]),
               2: (vecs[:, 48:49], vecs[0:64, 49:50])}

        # persistent activations
        q1p = glob.tile([128, 2, NQ], BF16, tag="q1p")
        q2p = glob.tile([128, 2, NQ], BF16, tag="q2p")
        k1p = glob.tile([128, 2, N1], BF16, tag="k1p")
        k2p = glob.tile([128, 2, N], BF16, tag="k2p")
        v1n = glob.tile([128, N1 // 128, 196], BF16, tag="v1n")
        v2n = glob.tile([128, N // 128, 196], BF16, tag="v2n")
        for vn in (v1n, v2n):
            for h in range(4):
                nc.vector.memset(vn[:, :, h * 49 + 48:h * 49 + 49], 1.0)
        # padded dwconv input buffers (zeroed halo; interior written by kv evict)
        vp = {}
        for br, P in ((2, 18), (1, 10)):
            a = glob.tile([128, P * P * P], BF16, tag=f"vp{br}a")
            b = glob.tile([64, P * P * P], BF16, tag=f"vp{br}b")
            nc.vector.memset(a, 0.0)
            nc.vector.memset(b, 0.0)
            vp[br] = (a, b)
        xct_sb = glob.tile([128, CT, NQ], F32, tag="xct")

        # ------------------------------------------------------------------
        # LayerNorm machinery (all on-chip).
        # stats: per 512-chunk ones-matmul -> st[0]=sum, st[1]=sumsq (PSUM)
        # -> SBUF -> PE-transpose into [tok(part), 2] -> vector math ->
        # PE-transpose a/b back to rows -> abrow SBUF [2, 512] f32.
        # Returns list of abrow tiles (one per 512-chunk).
        # ------------------------------------------------------------------
        def ln_rows(src, ntok, key, pstat, paux, src_f32=False):
            nch = ntok // 512
            K = ntok // 128
            ones = ones_f if src_f32 else ones_b
            sq_dt = F32 if src_f32 else BF16
            sxq = rows.tile([128, 2 * K], F32, tag="sxq", name=f"sxq_{key}")
            for ch in range(nch):
                st = pstat.tile([33, 512], F32, tag="st")
                for ct in range(CT):
                    r = src(ct)[:, ch * 512:(ch + 1) * 512]
                    nc.tensor.matmul(st[0:1, :], lhsT=ones, rhs=r,
                                     start=(ct == 0), stop=(ct == CT - 1))
                for ct in range(CT):
                    sqt = rows.tile([128, 512], sq_dt, tag=f"sqc{int(src_f32)}")
                    nc.scalar.activation(out=sqt, in_=src(ct)[:, ch * 512:(ch + 1) * 512],
                                         func=AF.Square)
                    nc.tensor.matmul(st[32:33, :], lhsT=ones, rhs=sqt,
                                     start=(ct == 0), stop=(ct == CT - 1))
                sts = rows.tile([33, 512], F32, tag="sts")
                nc.vector.tensor_copy(out=sts, in_=st)
                tps = paux.tile([128, 512], F32, tag="tps")
                for j in range(4):
                    nc.tensor.transpose(tps[:, 33 * j:33 * j + 33],
                                        sts[:, j * 128:(j + 1) * 128], identf[0:33, 0:33])
                tview = bass.AP(tensor=tps.tensor, offset=tps.offset,
                                ap=[list(tps.ap[0]), [33, 4], [32, 2]])
                nc.vector.tensor_copy(out=sxq[:, 8 * ch:8 * ch + 8], in_=tview)
            # vector math on [128, K] (strided views of sxq)
            sx = sxq[:, 0:2 * K].rearrange("p (k two) -> p two k", two=2)
            rm = rows.tile([128, K], F32, tag="rm")
            rq = rows.tile([128, K], F32, tag="rq")
            nc.vector.tensor_scalar_mul(out=rm, in0=sx[:, 0], scalar1=1.0 / C)
            nc.vector.tensor_scalar_mul(out=rq, in0=sx[:, 1], scalar1=1.0 / C)
            rv = rows.tile([128, K], F32, tag="rv")
            nc.vector.tensor_mul(rv, rm, rm)
            nc.vector.tensor_sub(rv, rq, rv)
            nc.scalar.activation(out=rv, in_=rv, func=AF.Sqrt, bias=eps_t)
            ab2 = rows.tile([128, 2 * K], F32, tag="ab2")
            ab2v = ab2.rearrange("p (k two) -> p two k", two=2)
            nc.vector.reciprocal(out=ab2v[:, 0], in_=rv)
            nc.vector.scalar_tensor_tensor(out=ab2v[:, 1], in0=rm, scalar=-1.0,
                                           in1=ab2v[:, 0], op0=AL.mult, op1=AL.mult)
            abrows = []
            for ch in range(nch):
                rab = paux.tile([128, 512], F32, tag="rab")
                for j in range(4):
                    nc.tensor.transpose(rab[0:2, j * 128:(j + 1) * 128],
                                        ab2[:, (4 * ch + j) * 2:(4 * ch + j) * 2 + 2],
                                        identf)
                abrow = rows.tile([2, 512], BF16, tag="abrow", name=f"abrow_{key}{ch}")
                nc.vector.tensor_copy(out=abrow, in_=rab[0:2, :])
                abrows.append(abrow)
            return abrows

        def ln_bcast(abrow, pab):
            """abrow [2,512] f32 -> A,B broadcast PSUM tiles [128,512] each."""
            ab = pab.tile([128, 1024], F32, tag="ab")
            nc.tensor.matmul(ab[:, 0:512], lhsT=sel2[:, 0, :],
                             rhs=abrow[0:2, :], start=True, stop=True)
            nc.tensor.matmul(ab[:, 512:1024], lhsT=sel2[:, 1, :],
                             rhs=abrow[0:2, :], start=True, stop=True)
            return ab

        # ============ Phase A: LN1, projections, convs ============
        def ln_bsb(ab):
            """evict the B-broadcast half to SBUF (Pool cannot read PSUM)."""
            bsb = rows.tile([128, 512], BF16, tag="bsb")
            nc.vector.tensor_copy(out=bsb, in_=ab[:, 512:1024])
            return bsb

        def ln_apply(src_sl, dst_sl, ab, bsb):
            """dst = src * A + B; mul on DVE (PSUM A), add on Pool (SBUF B)."""
            tmp = rows.tile([128, 512], BF16, tag="lnapp")
            nc.vector.tensor_mul(tmp, src_sl, ab[:, 0:512])
            nc.gpsimd.tensor_add(dst_sl, tmp, bsb)

        with tc.tile_pool(name="big", bufs=1) as big, \
             tc.tile_pool(name="pmm", bufs=2, space="PSUM") as pmm, \
             tc.tile_pool(name="pab", bufs=1, space="PSUM") as pab, \
             tc.tile_pool(name="pstat", bufs=1, space="PSUM") as pstat, \
             tc.tile_pool(name="paux", bufs=1, space="PSUM") as paux:

            xt_sb = big.tile([128, CT, N], BF16, tag="t_big1", name="xt_sb")
            dma(out=xt_sb, in_=d["xt"].rearrange("(t p) n -> p t n", p=128))
            dma(out=xct_sb, in_=d["xct"].rearrange("(t p) n -> p t n", p=128))
            wcat_sb = big.tile([128, CT, 1920], BF16, tag="t_wcat", name="wcat_sb")
            dma(out=wcat_sb, in_=d["wcat"].rearrange("(t p) co -> p t co", p=128))
            qw_sb = wcat_sb[:, :, 0:512]
            kv2w_sb = wcat_sb[:, :, 512:1024]
            kv1w_sb = wcat_sb[:, :, 1024:1536]
            s2w_sb = wcat_sb[:, :, 1536:1920]
            ab1 = ln_rows(lambda ct: xt_sb[:, ct, :], N, "l1", pstat, paux)
            xa = big.tile([128, CT, N], BF16, tag="t_big2", name="xa")
            for ch in range(N // 512):
                ab = ln_bcast(ab1[ch], pab)
                bsb = ln_bsb(ab)
                for ct in range(CT):
                    sl = slice(ch * 512, (ch + 1) * 512)
                    ln_apply(xt_sb[:, ct, sl], xa[:, ct, sl], ab, bsb)

            # --- q projection: own query chunk gets its own LN from xct ---
            ablq = ln_rows(lambda ct: xct_sb[:, ct, :], NQ, "lq", pstat, paux,
                           src_f32=True)
            xaq = big.tile([128, CT, NQ], BF16, tag="t_y1", name="xaq")
            for ch in range(NQ // 512):
                ab = ln_bcast(ablq[ch], pab)
                bsb = ln_bsb(ab)
                for ct in range(CT):
                    sl = slice(ch * 512, (ch + 1) * 512)
                    ln_apply(xct_sb[:, ct, sl], xaq[:, ct, sl], ab, bsb)
            for mt in range(4):
                qdst = q1p if mt < 2 else q2p
                tt = mt % 2
                for ch in range(NQ // 512):
                    ps = pmm.tile([128, 512], F32, tag="mm")
                    for ct in range(CT):
                        nc.tensor.matmul(
                            ps, lhsT=qw_sb[:, ct, mt * 128:(mt + 1) * 128],
                            rhs=xaq[:, ct, ch * 512:(ch + 1) * 512],
                            start=(ct == 0), stop=(ct == CT - 1))
                    nc.vector.tensor_scalar(out=qdst[:, tt, ch * 512:(ch + 1) * 512],
                                            in0=ps, scalar1=qb_sb[:, mt:mt + 1],
                                            scalar2=None, op0=AL.add)

            # --- sr2 (1x1x1 conv, full res) ---
            y2 = big.tile([128, CT, N], BF16, tag="t_big1", name="y2")
            for mt in range(CT):
                for ch in range(N // 512):
                    ps = pmm.tile([128, 512], F32, tag="mm")
                    for ct in range(CT):
                        nc.tensor.matmul(
                            ps, lhsT=s2w_sb[:, ct, mt * 128:(mt + 1) * 128],
                            rhs=xa[:, ct, ch * 512:(ch + 1) * 512],
                            start=(ct == 0), stop=(ct == CT - 1))
                    nc.vector.tensor_scalar(
                        out=y2[:, mt, ch * 512:(ch + 1) * 512],
                        in0=ps, scalar1=s2b_sb[:, mt:mt + 1],
                        scalar2=None, op0=AL.add)

            # --- sr1 (2x2x2 stride-2 conv) ---
            s1w_sb = big.tile([128, 24, C], BF16, tag="t_s1w", name="s1w_sb")
            dma(out=s1w_sb, in_=d["s1w"].rearrange("o (t p) m -> p (o t) m", p=128))
            y1 = big.tile([128, CT, N1], BF16, tag="t_y1", name="y1")
            for mt in range(CT):
                ps = pmm.tile([128, 512], F32, tag="mm")
                k = 0
                for oi in range(8):
                    a_, b_, c_ = oi // 4, (oi // 2) % 2, oi % 2
                    rhs0 = xa.rearrange(
                        "p t (h a w b d c) -> p t a b c h w d",
                        h=8, a=2, w=8, b=2, d=8, c=2)
                    for ct in range(CT):
                        nc.tensor.matmul(
                            ps, lhsT=s1w_sb[:, oi * 3 + ct, mt * 128:(mt + 1) * 128],
                            rhs=rhs0[:, ct, a_, b_, c_],
                            start=(k == 0), stop=(k == 23))
                        k += 1
                nc.vector.tensor_scalar(out=y1[:, mt, :], in0=ps,
                                        scalar1=s1b_sb[:, mt:mt + 1],
                                        scalar2=None, op0=AL.add)

            # --- n2 LN + gelu -> x2 ---
            abn2 = ln_rows(lambda ct: y2[:, ct, :], N, "n2", pstat, paux)
            x2 = big.tile([128, CT, N], BF16, tag="t_big2", name="x2")
            for ch in range(N // 512):
                ab = ln_bcast(abn2[ch], pab)
                bsb = ln_bsb(ab)
                for ct in range(CT):
                    sl = slice(ch * 512, (ch + 1) * 512)
                    tmpn = rows.tile([128, 512], BF16, tag="lnapp", name="tmpn")
                    nc.vector.tensor_mul(tmpn, y2[:, ct, sl], ab[:, 0:512])
                    nc.gpsimd.tensor_add(tmpn, tmpn, bsb)
                    nc.scalar.activation(out=x2[:, ct, sl], in_=tmpn, func=AF.Gelu,
                                         bias=n2b_sb[:, ct:ct + 1],
                                         scale=n2w_sb[:, ct:ct + 1])

            # --- n1 LN + gelu -> x1 ---
            abn1 = ln_rows(lambda ct: y1[:, ct, :], N1, "n1", pstat, paux)
            x1 = big.tile([128, CT, N1], BF16, tag="t_x1", name="x1")
            ab = ln_bcast(abn1[0], pab)
            bsb = ln_bsb(ab)
            for ct in range(CT):
                tm1 = rows.tile([128, 512], BF16, tag="lnapp", name="tm1")
                nc.vector.tensor_mul(tm1, y1[:, ct, :], ab[:, 0:512])
                nc.gpsimd.tensor_add(tm1, tm1, bsb)
                nc.scalar.activation(out=x1[:, ct, :], in_=tm1, func=AF.Gelu,
                                     bias=n1b_sb[:, ct:ct + 1],
                                     scale=n1w_sb[:, ct:ct + 1])

            # --- kv projections; v lands directly in padded dw buffers ---
            for (src, wsb, kp, br, S, P, ntok) in (
                    (x2, kv2w_sb, k2p, 2, 16, 18, N),
                    (x1, kv1w_sb, k1p, 1, 8, 10, N1)):
                vpa, vpb = vp[br]
                vpav = vpa.rearrange("p (h w d) -> p h w d", h=P, w=P, d=P)
                vpbv = vpb.rearrange("p (h w d) -> p h w d", h=P, w=P, d=P)
                hpc = 512 // (S * S)  # h-planes per 512 chunk
                for mt in range(4):
                    for ch in range(ntok // 512):
                        ps = pmm.tile([128, 512], F32, tag="mm")
                        for ct in range(CT):
                            nc.tensor.matmul(
                                ps, lhsT=wsb[:, ct, mt * 128:(mt + 1) * 128],
                                rhs=src[:, ct, ch * 512:(ch + 1) * 512],
                                start=(ct == 0), stop=(ct == CT - 1))
                        if mt < 2:
                            nc.vector.tensor_copy(
                                out=kp[:, mt, ch * 512:(ch + 1) * 512], in_=ps)
                        elif mt == 2:
                            nc.vector.tensor_copy(
                                out=vpav[:, 1 + ch * hpc:1 + (ch + 1) * hpc, 1:S + 1, 1:S + 1],
                                in_=ps)
                        else:
                            nc.vector.tensor_copy(
                                out=vpbv[:, 1 + ch * hpc:1 + (ch + 1) * hpc, 1:S + 1, 1:S + 1],
                                in_=ps[0:64, :])

        # ============ Phase B: depthwise conv -> transposed v49 tiles ======
        with tc.tile_pool(name="dwp", bufs=1) as dwp, \
             tc.tile_pool(name="accp", bufs=3) as accp, \
             tc.tile_pool(name="pd", bufs=2, space="PSUM") as pdp, \
             tc.tile_pool(name="ptr", bufs=3, space="PSUM") as ptr:
            dga_t = dwp.tile([128, 54, 128], BF16, tag="t_dga", name="dga_t")
            dma(out=dga_t, in_=d["dga"][:, :, :])
            dgb_t = dwp.tile([64, 54, 64], BF16, tag="t_dgb", name="dgb_t")
            dma(out=dgb_t, in_=d["dgb"][:, :, :])
            dgs = {"dg2a": dga_t[:, 0:27, :], "dg1a": dga_t[:, 27:54, :],
                   "dg2b": dgb_t[:, 0:27, :], "dg1b": dgb_t[:, 27:54, :]}
            offs = [(dz, dy, dx) for dz in range(3) for dy in range(3)
                    for dx in range(3)]
            for (br, vn, S, P) in (() if "nodw" in ABL else
                                   ((2, v2n, 16, 18), (1, v1n, 8, 10))):
                ntok = S * S * S
                vpa, vpb = vp[br]
                for (half, vt, np_) in (("a", vpa, 128), ("b", vpb, 64)):
                    dgt = dgs[f"dg{br}{half}"]
                    bia = lcb[br][0 if half == "a" else 1]
                    vtv = vt.rearrange("p (h w d) -> p h w d", h=P, w=P, d=P)
                    hrows = 512 // (S * S)
                    for ch in range(ntok // 512):
                        pd_ = pdp.tile([128, 512], F32, tag="mm", name="pd_")
                        for j, (dz, dy, dx) in enumerate(offs):
                            rhs = bass.AP(
                                tensor=vt.tensor,
                                offset=vt.offset + ch * hrows * P * P
                                + dz * P * P + dy * P + dx,
                                ap=[list(vt.ap[0]), [P * P, hrows],
                                    [P, S], [1, S]])
                            nc.tensor.matmul(pd_[0:np_, :], lhsT=dgt[:, j, :],
                                             rhs=rhs, start=(j == 0),
                                             stop=(j == 26))
                        acc = accp.tile([128, 512], BF16, tag="t_acc", name="acc")
                        for hp in range(hrows):
                            nc.vector.scalar_tensor_tensor(
                                out=acc[0:np_, hp * S * S:(hp + 1) * S * S],
                                in0=pd_[0:np_, hp * S * S:(hp + 1) * S * S],
                                scalar=bia,
                                in1=vtv[:, 1 + ch * hrows + hp, 1:S + 1, 1:S + 1],
                                op0=AL.add, op1=AL.add)
                        for sub in range(4):
                            mt = ch * 4 + sub
                            tp = ptr.tile([128, 128], BF16, tag="tp")
                            nc.tensor.transpose(tp[:, 0:np_],
                                                acc[0:np_, sub * 128:(sub + 1) * 128],
                                                ident[0:np_, 0:np_])
                            c0 = 0 if half == "a" else 128
                            for (soff, ln, dcol) in _v49_runs(c0, np_):
                                nc.vector.tensor_copy(
                                    out=vn[:, mt, dcol:dcol + ln],
                                    in_=tp[:, soff:soff + ln])

        # ============ Phase C/D: attention (+ interleaved MLP) =============
        # MLP work for query block nb is emitted as a closure list and
        # drained into the issue stream of attention for block nb+1.
        pG = ctx.enter_context(tc.tile_pool(name="pG", bufs=1))
        ocat = pG.tile([128, 4, NQ], BF16, tag="ocat")
        # proj pad rows must be finite (pad weight rows are zero); rows
        # 32:48 / 96:112 are re-written by attention later (32-aligned access)
        nc.vector.memset(ocat[32:64, :, :], 0.0)
        nc.vector.memset(ocat[96:128, :, :], 0.0)
        pw_sb = mat_sb("pw", 4, C, pG)
        f1w_sb = mat_sb("f1w", CT, HID, pG)
        f2w_sb = mat_sb("f2w", 12, C, pG)
        zt = pG.tile([128, CT, NQ], F32, tag="zt")
        h1 = pG.tile([128, 12, 512], BF16, tag="h1")
        xm = pG.tile([128, CT, 512], BF16, tag="xm")
        ots = pG.tile([128, 4, C], F32, tag="ots")

        with tc.tile_pool(name="pS", bufs=2, space="PSUM") as pS, \
             tc.tile_pool(name="pO", bufs=1, space="PSUM") as pO, \
             tc.tile_pool(name="pM", bufs=2, space="PSUM") as pM, \
             tc.tile_pool(name="pex", bufs=3) as pex, \
             tc.tile_pool(name="prec", bufs=2) as prec:

            def mlp_emit(nb):
                """Closure list computing proj+LN2+MLP+store for query block nb."""
                ops = []
                nsl = slice(nb * 512, (nb + 1) * 512)

                def proj_mt(mt):
                    def f():
                        ps = pM.tile([128, 512], F32, tag="mm")
                        for kt in range(4):
                            nc.tensor.matmul(ps, lhsT=pw_sb[:, kt, mt * 128:(mt + 1) * 128],
                                             rhs=ocat[:, kt, nsl],
                                             start=(kt == 0), stop=(kt == 3))
                        nc.vector.scalar_tensor_tensor(
                            out=zt[:, mt, nsl], in0=ps, scalar=pb_sb[:, mt:mt + 1],
                            in1=xct_sb[:, mt, nsl], op0=AL.add, op1=AL.add)
                    return f
                for mt in range(CT):
                    ops.append(proj_mt(mt))

                # LN2 (single 512-token chunk)
                def ln2a():
                    zbf = pG.tile([128, CT, 512], BF16, tag="zbf", name=f"zbf{nb}")
                    nc.vector.tensor_copy(out=zbf, in_=zt[:, :, nsl])
                    st = pM.tile([128, 512], F32, tag="mm", name="l2st")
                    for ct in range(CT):
                        nc.tensor.matmul(st[0:1, :], lhsT=ones_b, rhs=zbf[:, ct, :],
                                         start=(ct == 0), stop=(ct == CT - 1))
                    for ct in range(CT):
                        sqt = rows.tile([128, 512], BF16, tag="sqc")
                        nc.scalar.activation(out=sqt, in_=zbf[:, ct, :], func=AF.Square)
                        nc.tensor.matmul(st[32:33, :], lhsT=ones_b, rhs=sqt,
                                         start=(ct == 0), stop=(ct == CT - 1))
                    sts = rows.tile([33, 512], F32, tag="sts")
                    nc.vector.tensor_copy(out=sts, in_=st[0:33, :])
                    tps = pM.tile([128, 512], F32, tag="mm", name="l2tp")
                    for j in range(4):
                        nc.tensor.transpose(tps[:, 33 * j:33 * j + 33],
                                            sts[:, j * 128:(j + 1) * 128],
                                            identf[0:33, 0:33])
                    sxq = pG.tile([128, 8], F32, tag="sxq8", name=f"sxq_l2{nb}")
                    tview = bass.AP(tensor=tps.tensor, offset=tps.offset,
                                    ap=[list(tps.ap[0]), [33, 4], [32, 2]])
                    nc.vector.tensor_copy(out=sxq, in_=tview)
                    sx = sxq.rearrange("p (k two) -> p two k", two=2)
                    rm = pG.tile([128, 4], F32, tag="rm4")
                    rq = pG.tile([128, 4], F32, tag="rq4")
                    nc.vector.tensor_scalar_mul(out=rm, in0=sx[:, 0], scalar1=1.0 / C)
                    nc.vector.tensor_scalar_mul(out=rq, in0=sx[:, 1], scalar1=1.0 / C)
                    rv = pG.tile([128, 4], F32, tag="rv4")
                    nc.vector.tensor_mul(rv, rm, rm)
                    nc.vector.tensor_sub(rv, rq, rv)
                    nc.scalar.activation(out=rv, in_=rv, func=AF.Sqrt, bias=eps_t)
                    ab2 = pG.tile([128, 8], F32, tag="ab8", name=f"ab2_l2{nb}")
                    ab2v = ab2.rearrange("p (k two) -> p two k", two=2)
                    nc.vector.reciprocal(out=ab2v[:, 0], in_=rv)
                    nc.vector.scalar_tensor_tensor(out=ab2v[:, 1], in0=rm, scalar=-1.0,
                                                   in1=ab2v[:, 0], op0=AL.mult, op1=AL.mult)
                    rab = pM.tile([128, 512], F32, tag="mm", name="l2rab")
                    for j in range(4):
                        nc.tensor.transpose(rab[0:2, j * 128:(j + 1) * 128],
                                            ab2[:, 2 * j:2 * j + 2], identf)
                    abrow = rows.tile([2, 512], BF16, tag="abrow", name=f"abrow_l2{nb}")
                    nc.vector.tensor_copy(out=abrow, in_=rab[0:2, :])
                    # broadcast (two sequential single-bank PSUM tiles)
                    abpa = pM.tile([128, 512], F32, tag="mm", name="l2aba")
                    nc.tensor.matmul(abpa, lhsT=sel2[:, 0, :],
                                     rhs=abrow[0:2, :], start=True, stop=True)
                    abpb = pM.tile([128, 512], F32, tag="mm", name="l2abb")
                    nc.tensor.matmul(abpb, lhsT=sel2[:, 1, :],
                                     rhs=abrow[0:2, :], start=True, stop=True)
                    tmp3 = pG.tile([128, 512], F32, tag="tmp3")
                    for ct in range(CT):
                        nc.vector.tensor_mul(tmp3, zt[:, ct, nsl], abpa)
                        nc.vector.tensor_add(tmp3, tmp3, abpb)
                        nc.scalar.activation(out=xm[:, ct, :], in_=tmp3, func=AF.Identity,
                                             bias=ln2b_sb[:, ct:ct + 1],
                                             scale=ln2w_sb[:, ct:ct + 1])
                ops.append(ln2a)

                def fc1_mt(mt):
                    def f():
                        ps = pM.tile([128, 512], F32, tag="mm")
                        for ct in range(CT):
                            nc.tensor.matmul(ps, lhsT=f1w_sb[:, ct, mt * 128:(mt + 1) * 128],
                                             rhs=xm[:, ct, :],
                                             start=(ct == 0), stop=(ct == CT - 1))
                        nc.scalar.activation(out=h1[:, mt, :], in_=ps, func=AF.Gelu,
                                             bias=f1b_sb[:, mt:mt + 1])
                    return f
                for mt in range(12):
                    ops.append(fc1_mt(mt))

                def fc2_mt(mt):
                    def f():
                        ps = pM.tile([128, 512], F32, tag="mm")
                        for kt in range(12):
                            nc.tensor.matmul(ps, lhsT=f2w_sb[:, kt, mt * 128:(mt + 1) * 128],
                                             rhs=h1[:, kt, :],
                                             start=(kt == 0), stop=(kt == 11))
                        nc.vector.scalar_tensor_tensor(
                            out=zt[:, mt, nsl], in0=ps, scalar=f2b_sb[:, mt:mt + 1],
                            in1=zt[:, mt, nsl], op0=AL.add, op1=AL.add)
                    return f
                for mt in range(CT):
                    ops.append(fc2_mt(mt))

                def store_nt(nt):
                    def f():
                        for ct in range(CT):
                            tp2 = pM.tile([128, 512], F32, tag="mm", name="otp")
                            nc.tensor.transpose(
                                tp2[:, 0:128],
                                zt[:, ct, nb * 512 + nt * 128:nb * 512 + (nt + 1) * 128],
                                identf)
                            nc.vector.tensor_copy(out=ots[:, nt, ct * 128:(ct + 1) * 128],
                                                  in_=tp2[:, 0:128])
                    return f
                for nt in range(4):
                    ops.append(store_nt(nt))

                def store_dma():
                    dma(out=out_d[nb * 512:(nb + 1) * 512, :].rearrange(
                        "(nt p) c -> p nt c", p=128),
                        in_=ots)
                ops.append(store_dma)
                return ops

            def attention(nb, filler):
                """Attention for query block nb; drains filler closures into
                the issue stream (~1 per pipeline unit)."""
                def drain(k):
                    for _ in range(k):
                        if filler:
                            filler.pop(0)()
                for (br, kp, qp, vn, nmt) in ((2, k2p, q2p, v2n, N // 128),
                                              (1, k1p, q1p, v1n, N1 // 128)):
                    oa = pO.tile([128, 512], F32, tag="oa")
                    ob = pO.tile([128, 512], F32, tag="ob")
                    units = [(mt, g) for mt in range(nmt) for g in range(2)]

                    S_t = {}
                    ex_t = {}

                    def emit_S(u):
                        mt, g = units[u]
                        St = pS.tile([128, 1024], F32, tag="S", name=f"S{nb}{br}")
                        for r in range(2):
                            nc.tensor.matmul(
                                St[:, r * 512:(r + 1) * 512],
                                lhsT=kp[64 * r:64 * r + 64, g, mt * 128:(mt + 1) * 128],
                                rhs=qp[64 * r:64 * r + 64, g, nb * 512:(nb + 1) * 512],
                                start=True, stop=True, tile_position=(64 * r, 0))
                        ex = pex.tile([128, 1024], BF16, tag="ex")
                        nc.scalar.activation(out=ex, in_=St, func=AF.Exp)
                        S_t[u] = St
                        ex_t[u] = ex

                    def emit_AV(u):
                        mt, g = units[u]
                        ot = oa if g == 0 else ob
                        ex = ex_t.pop(u)
                        S_t.pop(u)
                        for r in range(2):
                            h = 2 * g + r
                            nc.tensor.matmul(ot[64 * r:64 * r + 49, :],
                                             lhsT=vn[:, mt, h * 49:(h + 1) * 49],
                                             rhs=ex[:, r * 512:(r + 1) * 512],
                                             start=(mt == 0), stop=(mt == nmt - 1),
                                             tile_position=(0, 64 * r))

                    emit_S(0)
                    for u in range(len(units)):
                        if u + 1 < len(units):
                            emit_S(u + 1)
                        emit_AV(u)
                        drain(1)

                    # denominators (rows 48/112 of oa/ob -> den rows 16/48/80/112
                    # via 32-row block copies) -> reciprocal -> broadcast
                    den = prec.tile([128, 512], F32, tag="den")
                    nc.vector.tensor_copy(out=den[0:32, :], in_=oa[32:64, :])
                    nc.vector.tensor_copy(out=den[32:64, :], in_=oa[96:128, :])
                    nc.vector.tensor_copy(out=den[64:96, :], in_=ob[32:64, :])
                    nc.vector.tensor_copy(out=den[96:128, :], in_=ob[96:128, :])
                    # reciprocal per 32-row block; the broadcast matmuls
                    # contract only K=17 rows (0..16) so no stale PSUM rows
                    # (49-63/113-127 of oa/ob, never written by AV) are read:
                    # stale Inf/NaN times a zero selector weight would poison
                    # the output.
                    rcs = []
                    for blk in range(4):
                        rct = prec.tile([32, 512], F32, tag=f"rc{blk}")
                        nc.vector.reciprocal(out=rct,
                                             in_=den[32 * blk:32 * blk + 32, :])
                        rcs.append(rct)
                    sel17 = sel64t[0:17, 0:48]
                    tbase = 0 if br == 1 else 2
                    for pi, srcp in enumerate((oa, ob)):
                        rc = pS.tile([128, 1024], F32, tag="S", name=f"rc{nb}{br}{pi}")
                        nc.tensor.matmul(rc[0:48, 0:512], lhsT=sel17,
                                         rhs=rcs[2 * pi][0:17, :],
                                         start=True, stop=True)
                        nc.tensor.matmul(rc[64:112, 0:512], lhsT=sel17,
                                         rhs=rcs[2 * pi + 1][0:17, :],
                                         start=True, stop=True,
                                         tile_position=(0, 64))
                        recb = pex.tile([128, 512], F32, tag="recb")
                        nc.vector.tensor_copy(out=recb[0:112, :], in_=rc[0:112, 0:512])
                        tt = tbase + pi
                        for r in range(2):
                            nc.vector.tensor_mul(
                                ocat[64 * r:64 * r + 48, tt, nb * 512:(nb + 1) * 512],
                                srcp[64 * r:64 * r + 48, :], recb[64 * r:64 * r + 48, :])
                    drain(4)

            if "noattn" not in ABL:
                attention(0, [])
                fill = mlp_emit(0)
                attention(1, fill)
                for f in fill:
                    f()
                for f in mlp_emit(1):
                    f()


_PROG = None


def _get_program():
    global _PROG
    if _PROG is None:
        _PROG = build_program()
    return _PROG


def _diag(w):
    """[n, 27] weights -> [n, 27, n] per-offset diagonal matrices (bf16)."""
    n = w.shape[0]
    out = np.zeros((n, 27, n), BF)
    idx = np.arange(n)
    for j in range(27):
        out[idx, j, idx] = w[:, j].astype(BF)
    return out


def _pad_heads_out(w384, bias=None):
    """[C_in, 384] head-major (8x48) -> [C_in, 512] with 64-row head slots."""
    cin = w384.shape[0]
    out = np.zeros((cin, 512), np.float32)
    bout = np.zeros(512, np.float32)
    for h in range(8):
        tt, e = h // 2, h % 2
        out[:, 128 * tt + 64 * e:128 * tt + 64 * e + 48] = w384[:, h * 48:(h + 1) * 48]
        if bias is not None:
            bout[128 * tt + 64 * e:128 * tt + 64 * e + 48] = bias[h * 48:(h + 1) * 48]
    return out, bout


def _pad_kv(wT):
    """kv weight [C_in, 384] (k 4x48 | v 192) -> [C_in, 512]:
    tiles 0,1 = k head slots, tile 2 = v 0:128, tile 3 = v 128:192 + pad."""
    cin = wT.shape[0]
    out = np.zeros((cin, 512), np.float32)
    for h in range(4):
        tt, e = h // 2, h % 2
        out[:, 128 * tt + 64 * e:128 * tt + 64 * e + 48] = wT[:, h * 48:(h + 1) * 48]
    out[:, 256:384] = wT[:, 192:320]
    out[:, 384:448] = wT[:, 320:384]
    return out


def kernel(x, ln1_w, ln1_b, q_w, sr1_w, sr1_b, n1_w, n1_b, sr2_w, sr2_b,
           n2_w, n2_b, kv1_w, kv2_w, lc1_w, lc1_b, lc2_w, lc2_b,
           proj_w, proj_b, ln2_w, ln2_b, fc1_w, fc1_b, fc2_w, fc2_b,
           H, W, D):
    f = lambda a: np.asarray(a, np.float32)
    x = f(x)
    ln1_w, ln1_b = f(ln1_w), f(ln1_b)
    qs = HD ** -0.5
    lc1 = f(lc1_w).reshape(C2, 27)
    lc2 = f(lc2_w).reshape(C2, 27)

    qwp, qbp = _pad_heads_out((f(q_w) * ln1_w[None, :]).T * qs,
                              f(q_w) @ ln1_b * qs)
    # proj: padded input rows (512) matching ocat head-slot layout
    pwp = np.zeros((512, C), np.float32)
    pT = f(proj_w).T  # [384 in, 384 out]
    for g in range(8):
        tt, e = g // 2, g % 2
        pwp[128 * tt + 64 * e:128 * tt + 64 * e + 48, :] = pT[g * 48:(g + 1) * 48, :]

    # pack all bias/affine vectors into one [128, 50] f32 tensor
    def pack_pt(v, ncol):
        return f(v).reshape(ncol, 128).T

    vecs = np.zeros((128, 50), np.float32)
    vecs[:, 0:4] = pack_pt(qbp, 4)
    vecs[:, 4:7] = pack_pt(f(sr2_b) + f(sr2_w)[:, :, 0, 0, 0] @ ln1_b, 3)
    vecs[:, 7:10] = pack_pt(f(sr1_b) + np.einsum("ocijk,c->o", f(sr1_w), ln1_b), 3)
    vecs[:, 10:13] = pack_pt(n1_w, 3)
    vecs[:, 13:16] = pack_pt(n1_b, 3)
    vecs[:, 16:19] = pack_pt(n2_w, 3)
    vecs[:, 19:22] = pack_pt(n2_b, 3)
    vecs[:, 22:25] = pack_pt(proj_b, 3)
    vecs[:, 25:28] = pack_pt(ln2_w, 3)
    vecs[:, 28:31] = pack_pt(ln2_b, 3)
    vecs[:, 31:34] = pack_pt(fc2_b, 3)
    vecs[:, 34:46] = pack_pt(fc1_b, 12)
    vecs[:, 46] = f(lc1_b)[0:128]
    vecs[0:64, 47] = f(lc1_b)[128:192]
    vecs[:, 48] = f(lc2_b)[0:128]
    vecs[0:64, 49] = f(lc2_b)[128:192]

    wcat = np.concatenate([
        qwp,
        _pad_kv(f(kv2_w).T),
        _pad_kv(f(kv1_w).T),
        (f(sr2_w)[:, :, 0, 0, 0] * ln1_w[None, :]).T,
    ], axis=1)

    sel2 = np.zeros((2, 2, 128), np.float32)
    sel2[0, 0, :] = 1.0
    sel2[1, 1, :] = 1.0
    sel64 = np.zeros((64, 2, 48), np.float32)
    sel64[16, 0, :] = 1.0
    sel64[48, 1, :] = 1.0

    wm = {
        "vecs": vecs,
        "sel2": sel2.reshape(2, 256).astype(BF),
        "sel64": sel64.reshape(64, 96),
        "wcat": np.ascontiguousarray(wcat).astype(BF),
        "s1w": np.ascontiguousarray(
            (f(sr1_w) * ln1_w[None, :, None, None, None])
            .transpose(2, 3, 4, 1, 0).reshape(8, C, C)).astype(BF),
        "dga": np.concatenate([_diag(lc2[0:128]), _diag(lc1[0:128])], axis=1),
        "dgb": np.concatenate([_diag(lc2[128:192]), _diag(lc1[128:192])], axis=1),
        "pw": np.ascontiguousarray(pwp).astype(BF),
        "f1w": np.ascontiguousarray(f(fc1_w).T).astype(BF),
        "f2w": np.ascontiguousarray(f(fc2_w).T).astype(BF),
    }

    in_maps = []
    for core in range(8):
        b, qc = core // 4, core % 4
        xtb = x[b].T
        m = dict(wm)
        m["xt"] = np.ascontiguousarray(xtb).astype(BF)
        m["xct"] = np.ascontiguousarray(xtb[:, qc * NQ:(qc + 1) * NQ]).astype(np.float32)
        in_maps.append(m)

    nc = _get_program()
    res = run_bass_kernel_spmd(nc, in_maps, list(range(8)))

    out = np.empty((B, N, C), np.float32)
    for core in range(8):
        b, qc = core // 4, core % 4
        out[b, qc * NQ:(qc + 1) * NQ, :] = res.results[core]["out"]
    return out


# revision 51
# speedup vs baseline: 1.0470x; 1.0470x over previous
"""Trainium2 Bass kernel for nn_Block_11166914969721 (dense transformer block).

Sharding: 8 cores = (batch b in {0,1}) x (query chunk qc in {0..3}, 1024
queries each). Each core recomputes the full KV side for its batch and
computes attention + proj + MLP for its own query chunk.

Key implementation points (v2, low-DMA):
- All activations in T-layout [channels(part), tokens(free)].
- Projection weights are host-padded into 64-row head slots (zeros in the
  pad rows) so each 128-row PSUM co-tile evicts with a single vector op
  directly into the persistent q/k/ocat layouts -- no scatter DMAs.
- LayerNorm stats stay on chip: ones-matmul column sums -> PE transpose to
  [tokens(part), 2] -> vector math -> PE transpose back to rows -> K=1
  f32r ones-matmul broadcast to [128, tok] PSUM tiles.
- Depthwise 3x3x3 conv = 27 PSUM-accumulated diag matmuls on shifted views
  of a zero-padded buffer; kv-proj evictions write the padded buffer
  interior directly. Transposed 49-augmented V tiles are written with
  direct PSUM->SBUF copies.
- Attention is software-pipelined: S(u+1) is issued before AV(u) so the
  Act engine's exp stream never starves; softmax denominators come from an
  appended ones-column on V, inverted on chip and broadcast with K=1
  matmuls.
- The MLP for query block 0 is issue-interleaved under attention of query
  block 1.
"""

import os
import numpy as np
import ml_dtypes

import concourse.bass as bass
import concourse.mybir as mybir
import concourse.tile as tile
from concourse.bass_utils import run_bass_kernel_spmd
from concourse.masks import make_identity
from concourse.vector_clock import ScopedClock

BF = ml_dtypes.bfloat16
AL = mybir.AluOpType
AF = mybir.ActivationFunctionType
F32 = mybir.dt.float32
F32R = mybir.dt.float32r
BF16 = mybir.dt.bfloat16

# ---------------------------------------------------------------------------
# Workarounds: walrus in this container accepts at most ONE sem-wait per
# instruction. (a) Tile's kernel-tail drain aggregates one wait per live
# proc -> spread across SP nops. (b) Mid-kernel instructions may also get
# several waits -> post-pass splits them onto same-engine NoOps.
# ---------------------------------------------------------------------------


def _patched_drain_and_barrier(self, tick_clock, wait_clock):
    nc = self.nc
    collector = nc.sync.nop(nofuse=True)
    wait_clock.add_sem_waits(collector.ins, ScopedClock({None: tick_clock.global_clock}))
    si = collector.ins.sync_info
    waits = list(si.on_wait) if si is not None and si.on_wait else []
    if si is not None:
        si.on_wait = waits[:1]
    for i in range(1, len(waits)):
        nop = nc.sync.nop(nofuse=True)
        nop.ins.sync_info = mybir.SyncInfo(on_wait=waits[i:i + 1], on_update=[])
    nc.sync.drain()
    nc.all_engine_barrier()
    assert self.sems is not None
    popped = nc._tile_sem_poison_stack.pop()
    assert popped is self._sem_poison
    nc.clear_and_free_semaphores(list(self.sems.allocated().values()))
    nc.all_engine_barrier()


tile.TileContext._drain_and_barrier = _patched_drain_and_barrier


def _split_multi_waits(nc):
    cnt = 0
    for fn in nc.m.functions:
        for bb in fn.blocks:
            out = []
            for inst in bb.instructions:
                si = inst.sync_info
                if si is not None and si.on_wait and len(si.on_wait) > 1:
                    waits = list(si.on_wait)
                    for w in waits[:-1]:
                        cnt += 1
                        out.append(mybir.InstNoOp(
                            name=f"nwsplit{cnt}",
                            engine=inst.engine,
                            sync_info=mybir.SyncInfo(on_wait=[w], on_update=[]),
                            bass_nofuse=True))
                    si.on_wait = waits[-1:]
                out.append(inst)
            bb.instructions[:] = out
    return cnt


# ---------------------------------------------------------------------------
B, N, C = 2, 4096, 384
HD = 48
C2 = 192
N1 = 512
HID = 4 * C
NQ = 1024          # queries per core
CT = 3             # channel tiles of 128
EPS = 1e-5
ABL = os.environ.get("KABL", "")


def _v49_runs(c0, ln):
    """channel range of v -> 49-augmented column offsets: (src_off, len, dst_col)."""
    out = []
    bs = sorted(set([c0, c0 + ln] + [k * 48 for k in range(1, 4) if c0 < k * 48 < c0 + ln]))
    for a, b in zip(bs, bs[1:]):
        out.append((a - c0, b - a, (a // 48) * 49 + a % 48))
    return out


def build_program():
    nc = bass.Bass()
    d = {}

    def din(name, shape, dt):
        d[name] = nc.dram_tensor(name, shape, dt, kind="ExternalInput")

    din("xt", [C, N], BF16)
    din("xct", [C, NQ], F32)
    din("vecs", [128, 50], F32)     # all bias/affine vectors, pre-packed
    din("sel2", [2, 256], BF16)     # one-hot row selectors (K=2)
    din("sel64", [64, 96], F32)     # one-hot selectors rows 16/48 (K=64)
    din("wcat", [C, 1920], BF16)    # qw(512) | kv2w(512) | kv1w(512) | s2w(384)
    din("s1w", [8, C, C], BF16)
    din("dga", [128, 54, 128], BF16)  # dg2a | dg1a
    din("dgb", [64, 54, 64], BF16)    # dg2b | dg1b
    din("pw", [512, C], BF16)       # padded input rows
    din("f1w", [C, HID], BF16)
    din("f2w", [HID, C], BF16)

    out_d = nc.dram_tensor("out", [NQ, C], F32, kind="ExternalOutput")

    with tile.TileContext(nc, pool_alloc_mode="queue") as tc:
        _body(tc, nc, d, out_d)
    _split_multi_waits(nc)
    return nc


def _body(tc, nc, d, out_d):
    from contextlib import ExitStack

    dma = nc.gpsimd.dma_start

    ctx = ExitStack()
    with ctx:
        glob = ctx.enter_context(tc.tile_pool(name="glob", bufs=1))
        wpool = ctx.enter_context(tc.tile_pool(name="wpool", bufs=1))
        rows = ctx.enter_context(tc.tile_pool(name="rows", bufs=2))

        ones_b = glob.tile([128, 1], BF16, tag="ones_b")
        nc.vector.memset(ones_b, 1.0)
        ones_f = glob.tile([128, 1], F32, tag="ones_f")
        nc.vector.memset(ones_f, 1.0)
        onesrow_f = glob.tile([1, 128], F32, tag="onesrow_f")
        nc.vector.memset(onesrow_f, 1.0)
        # row-selector lhsT matrices (loaded: partition-offset memsets are
        # not legal engine ops): sel2[:, j, :] one-hot row j (K=2);
        # sel64[:, j, :] one-hot row 16/48 (K=64, for denominator rows)
        sel2t = glob.tile([2, 256], BF16, tag="sel2")
        dma(out=sel2t, in_=d["sel2"][:, :])
        sel2 = sel2t.rearrange("p (j c) -> p j c", j=2)
        sel64t = glob.tile([64, 96], F32, tag="sel64")
        dma(out=sel64t, in_=d["sel64"][:, :])
        sel64 = sel64t.rearrange("p (j c) -> p j c", j=2)
        eps_t = glob.tile([128, 1], F32, tag="eps")
        nc.vector.memset(eps_t, EPS)
        ident = glob.tile([128, 128], BF16, tag="ident")
        make_identity(nc, ident)
        identf = glob.tile([128, 128], F32, tag="identf")
        make_identity(nc, identf)

        def mat_sb(name, ktiles, cols, pool, tag=None):
            t = pool.tile([128, ktiles, cols], BF16, tag=tag or f"m_{name}", name=name)
            dma(out=t, in_=d[name].rearrange("(t p) co -> p t co", p=128))
            return t

        vecs = wpool.tile([128, 50], F32, tag="vecs")
        dma(out=vecs, in_=d["vecs"][:, :])
        qb_sb = vecs[:, 0:4]
        s2b_sb = vecs[:, 4:7]
        s1b_sb = vecs[:, 7:10]
        n1w_sb = vecs[:, 10:13]
        n1b_sb = vecs[:, 13:16]
        n2w_sb = vecs[:, 16:19]
        n2b_sb = vecs[:, 19:22]
        pb_sb = vecs[:, 22:25]
        ln2w_sb = vecs[:, 25:28]
        ln2b_sb = vecs[:, 28:31]
        f2b_sb = vecs[:, 31:34]
        f1b_sb = vecs[:, 34:46]
        lcb = {1: (vecs[:, 46:47], vecs[0:64, 47:48]),
               2: (vecs[:, 48:49], vecs[0:64, 49:50])}

        # persistent activations
        q1p = glob.tile([128, 2, NQ], BF16, tag="q1p")
        q2p = glob.tile([128, 2, NQ], BF16, tag="q2p")
        k1p = glob.tile([128, 2, N1], BF16, tag="k1p")
        k2p = glob.tile([128, 2, N], BF16, tag="k2p")
        v1n = glob.tile([128, N1 // 128, 196], BF16, tag="v1n")
        v2n = glob.tile([128, N // 128, 196], BF16, tag="v2n")
        for vn in (v1n, v2n):
            for h in range(4):
                nc.vector.memset(vn[:, :, h * 49 + 48:h * 49 + 49], 1.0)
        # padded dwconv input buffers (zeroed halo; interior written by kv evict)
        vp = {}
        for br, P in ((2, 18), (1, 10)):
            a = glob.tile([128, P * P * P], BF16, tag=f"vp{br}a")
            b = glob.tile([64, P * P * P], BF16, tag=f"vp{br}b")
            nc.vector.memset(a, 0.0)
            nc.vector.memset(b, 0.0)
            vp[br] = (a, b)
        xct_sb = glob.tile([128, CT, NQ], F32, tag="xct")

        # ------------------------------------------------------------------
        # LayerNorm machinery (all on-chip).
        # stats: per 512-chunk ones-matmul -> st[0]=sum, st[1]=sumsq (PSUM)
        # -> SBUF -> PE-transpose into [tok(part), 2] -> vector math ->
        # PE-transpose a/b back to rows -> abrow SBUF [2, 512] f32.
        # Returns list of abrow tiles (one per 512-chunk).
        # ------------------------------------------------------------------
        def ln_rows(src, ntok, key, pstat, paux, src_f32=False):
            nch = ntok // 512
            K = ntok // 128
            ones = ones_f if src_f32 else ones_b
            sxq = rows.tile([128, 2 * K], F32, tag="sxq", name=f"sxq_{key}")
            for ch in range(nch):
                st = pstat.tile([33, 512], F32, tag="st")
                for ct in range(CT):
                    r = src(ct)[:, ch * 512:(ch + 1) * 512]
                    nc.tensor.matmul(st[0:1, :], lhsT=ones, rhs=r,
                                     start=(ct == 0), stop=(ct == CT - 1))
                for ct in range(CT):
                    sqt = rows.tile([128, 512], BF16, tag="sqc0")
                    nc.scalar.activation(out=sqt, in_=src(ct)[:, ch * 512:(ch + 1) * 512],
                                         func=AF.Square)
                    nc.tensor.matmul(st[32:33, :], lhsT=ones_b, rhs=sqt,
                                     start=(ct == 0), stop=(ct == CT - 1))
                sts = rows.tile([33, 512], F32, tag="sts")
                nc.vector.tensor_copy(out=sts, in_=st)
                tps = paux.tile([128, 512], F32, tag="tps")
                for j in range(4):
                    nc.tensor.transpose(tps[:, 33 * j:33 * j + 33],
                                        sts[:, j * 128:(j + 1) * 128], identf[0:33, 0:33])
                tview = bass.AP(tensor=tps.tensor, offset=tps.offset,
                                ap=[list(tps.ap[0]), [33, 4], [32, 2]])
                nc.vector.tensor_copy(out=sxq[:, 8 * ch:8 * ch + 8], in_=tview)
            # vector math on [128, K] (strided views of sxq)
            sx = sxq[:, 0:2 * K].rearrange("p (k two) -> p two k", two=2)
            rm = rows.tile([128, K], F32, tag="rm")
            rq = rows.tile([128, K], F32, tag="rq")
            nc.vector.tensor_scalar_mul(out=rm, in0=sx[:, 0], scalar1=1.0 / C)
            nc.vector.tensor_scalar_mul(out=rq, in0=sx[:, 1], scalar1=1.0 / C)
            rv = rows.tile([128, K], F32, tag="rv")
            nc.vector.tensor_mul(rv, rm, rm)
            nc.vector.tensor_sub(rv, rq, rv)
            nc.scalar.activation(out=rv, in_=rv, func=AF.Sqrt, bias=eps_t)
            ab2 = rows.tile([128, 2 * K], F32, tag="ab2")
            ab2v = ab2.rearrange("p (k two) -> p two k", two=2)
            nc.vector.reciprocal(out=ab2v[:, 0], in_=rv)
            nc.vector.scalar_tensor_tensor(out=ab2v[:, 1], in0=rm, scalar=-1.0,
                                           in1=ab2v[:, 0], op0=AL.mult, op1=AL.mult)
            abrows = []
            for ch in range(nch):
                rab = paux.tile([128, 512], F32, tag="rab")
                for j in range(4):
                    nc.tensor.transpose(rab[0:2, j * 128:(j + 1) * 128],
                                        ab2[:, (4 * ch + j) * 2:(4 * ch + j) * 2 + 2],
                                        identf)
                abrow = rows.tile([2, 512], BF16, tag="abrow", name=f"abrow_{key}{ch}")
                nc.vector.tensor_copy(out=abrow, in_=rab[0:2, :])
                abrows.append(abrow)
            return abrows

        def ln_bcast(abrow, pab):
            """abrow [2,512] f32 -> A,B broadcast PSUM tiles [128,512] each."""
            ab = pab.tile([128, 1024], F32, tag="ab")
            nc.tensor.matmul(ab[:, 0:512], lhsT=sel2[:, 0, :],
                             rhs=abrow[0:2, :], start=True, stop=True)
            nc.tensor.matmul(ab[:, 512:1024], lhsT=sel2[:, 1, :],
                             rhs=abrow[0:2, :], start=True, stop=True)
            return ab

        # ============ Phase A: LN1, projections, convs ============
        def ln_bsb(ab):
            """evict the B-broadcast half to SBUF (Pool cannot read PSUM)."""
            bsb = rows.tile([128, 512], BF16, tag="bsb")
            nc.vector.tensor_copy(out=bsb, in_=ab[:, 512:1024])
            return bsb

        def ln_apply(src_sl, dst_sl, ab, bsb):
            """dst = src * A + B; mul on DVE (PSUM A), add on Pool (SBUF B)."""
            tmp = rows.tile([128, 512], BF16, tag="lnapp")
            nc.vector.tensor_mul(tmp, src_sl, ab[:, 0:512])
            nc.gpsimd.tensor_add(dst_sl, tmp, bsb)

        with tc.tile_pool(name="big", bufs=1) as big, \
             tc.tile_pool(name="pmm", bufs=2, space="PSUM") as pmm, \
             tc.tile_pool(name="pab", bufs=1, space="PSUM") as pab, \
             tc.tile_pool(name="pstat", bufs=2, space="PSUM") as pstat, \
             tc.tile_pool(name="paux", bufs=1, space="PSUM") as paux:

            xt_sb = big.tile([128, CT, N], BF16, tag="t_big1", name="xt_sb")
            dma(out=xt_sb, in_=d["xt"].rearrange("(t p) n -> p t n", p=128))
            dma(out=xct_sb, in_=d["xct"].rearrange("(t p) n -> p t n", p=128))
            wcat_sb = big.tile([128, CT, 1920], BF16, tag="t_wcat", name="wcat_sb")
            dma(out=wcat_sb, in_=d["wcat"].rearrange("(t p) co -> p t co", p=128))
            qw_sb = wcat_sb[:, :, 0:512]
            kv2w_sb = wcat_sb[:, :, 512:1024]
            kv1w_sb = wcat_sb[:, :, 1024:1536]
            s2w_sb = wcat_sb[:, :, 1536:1920]
            ab1 = ln_rows(lambda ct: xt_sb[:, ct, :], N, "l1", pstat, paux)
            xa = big.tile([128, CT, N], BF16, tag="t_big2", name="xa")
            for ch in range(N // 512):
                ab = ln_bcast(ab1[ch], pab)
                bsb = ln_bsb(ab)
                for ct in range(CT):
                    sl = slice(ch * 512, (ch + 1) * 512)
                    ln_apply(xt_sb[:, ct, sl], xa[:, ct, sl], ab, bsb)

            # --- q projection: own query chunk gets its own LN from xct ---
            ablq = ln_rows(lambda ct: xct_sb[:, ct, :], NQ, "lq", pstat, paux,
                           src_f32=True)
            xaq = big.tile([128, CT, NQ], BF16, tag="t_y1", name="xaq")
            for ch in range(NQ // 512):
                ab = ln_bcast(ablq[ch], pab)
                bsb = ln_bsb(ab)
                for ct in range(CT):
                    sl = slice(ch * 512, (ch + 1) * 512)
                    ln_apply(xct_sb[:, ct, sl], xaq[:, ct, sl], ab, bsb)
            for mt in range(4):
                qdst = q1p if mt < 2 else q2p
                tt = mt % 2
                for ch in range(NQ // 512):
                    ps = pmm.tile([128, 512], F32, tag="mm")
                    for ct in range(CT):
                        nc.tensor.matmul(
                            ps, lhsT=qw_sb[:, ct, mt * 128:(mt + 1) * 128],
                            rhs=xaq[:, ct, ch * 512:(ch + 1) * 512],
                            start=(ct == 0), stop=(ct == CT - 1))
                    nc.vector.tensor_scalar(out=qdst[:, tt, ch * 512:(ch + 1) * 512],
                                            in0=ps, scalar1=qb_sb[:, mt:mt + 1],
                                            scalar2=None, op0=AL.add)

            # --- sr2 (1x1x1 conv, full res) ---
            y2 = big.tile([128, CT, N], BF16, tag="t_big1", name="y2")
            for mt in range(CT):
                for ch in range(N // 512):
                    ps = pmm.tile([128, 512], F32, tag="mm")
                    for ct in range(CT):
                        nc.tensor.matmul(
                            ps, lhsT=s2w_sb[:, ct, mt * 128:(mt + 1) * 128],
                            rhs=xa[:, ct, ch * 512:(ch + 1) * 512],
                            start=(ct == 0), stop=(ct == CT - 1))
                    nc.vector.tensor_scalar(
                        out=y2[:, mt, ch * 512:(ch + 1) * 512],
                        in0=ps, scalar1=s2b_sb[:, mt:mt + 1],
                        scalar2=None, op0=AL.add)

            # --- sr1 (2x2x2 stride-2 conv) ---
            s1w_sb = big.tile([128, 24, C], BF16, tag="t_s1w", name="s1w_sb")
            dma(out=s1w_sb, in_=d["s1w"].rearrange("o (t p) m -> p (o t) m", p=128))
            y1 = big.tile([128, CT, N1], BF16, tag="t_y1", name="y1")
            for mt in range(CT):
                ps = pmm.tile([128, 512], F32, tag="mm")
                k = 0
                for oi in range(8):
                    a_, b_, c_ = oi // 4, (oi // 2) % 2, oi % 2
                    rhs0 = xa.rearrange(
                        "p t (h a w b d c) -> p t a b c h w d",
                        h=8, a=2, w=8, b=2, d=8, c=2)
                    for ct in range(CT):
                        nc.tensor.matmul(
                            ps, lhsT=s1w_sb[:, oi * 3 + ct, mt * 128:(mt + 1) * 128],
                            rhs=rhs0[:, ct, a_, b_, c_],
                            start=(k == 0), stop=(k == 23))
                        k += 1
                nc.vector.tensor_scalar(out=y1[:, mt, :], in0=ps,
                                        scalar1=s1b_sb[:, mt:mt + 1],
                                        scalar2=None, op0=AL.add)

            # --- n2 LN + gelu -> x2 ---
            abn2 = ln_rows(lambda ct: y2[:, ct, :], N, "n2", pstat, paux)
            x2 = big.tile([128, CT, N], BF16, tag="t_big2", name="x2")
            for ch in range(N // 512):
                ab = ln_bcast(abn2[ch], pab)
                bsb = ln_bsb(ab)
                for ct in range(CT):
                    sl = slice(ch * 512, (ch + 1) * 512)
                    tmpn = rows.tile([128, 512], BF16, tag="lnapp", name="tmpn")
                    nc.vector.tensor_mul(tmpn, y2[:, ct, sl], ab[:, 0:512])
                    nc.gpsimd.tensor_add(tmpn, tmpn, bsb)
                    nc.scalar.activation(out=x2[:, ct, sl], in_=tmpn, func=AF.Gelu,
                                         bias=n2b_sb[:, ct:ct + 1],
                                         scale=n2w_sb[:, ct:ct + 1])

            # --- n1 LN + gelu -> x1 ---
            abn1 = ln_rows(lambda ct: y1[:, ct, :], N1, "n1", pstat, paux)
            x1 = big.tile([128, CT, N1], BF16, tag="t_x1", name="x1")
            ab = ln_bcast(abn1[0], pab)
            bsb = ln_bsb(ab)
            for ct in range(CT):
                tm1 = rows.tile([128, 512], BF16, tag="lnapp", name="tm1")
                nc.vector.tensor_mul(tm1, y1[:, ct, :], ab[:, 0:512])
                nc.gpsimd.tensor_add(tm1, tm1, bsb)
                nc.scalar.activation(out=x1[:, ct, :], in_=tm1, func=AF.Gelu,
                                     bias=n1b_sb[:, ct:ct + 1],
                                     scale=n1w_sb[:, ct:ct + 1])

            # --- kv projections; v lands directly in padded dw buffers ---
            for (src, wsb, kp, br, S, P, ntok) in (
                    (x2, kv2w_sb, k2p, 2, 16, 18, N),
                    (x1, kv1w_sb, k1p, 1, 8, 10, N1)):
                vpa, vpb = vp[br]
                vpav = vpa.rearrange("p (h w d) -> p h w d", h=P, w=P, d=P)
                vpbv = vpb.rearrange("p (h w d) -> p h w d", h=P, w=P, d=P)
                hpc = 512 // (S * S)  # h-planes per 512 chunk
                for mt in range(4):
                    for ch in range(ntok // 512):
                        ps = pmm.tile([128, 512], F32, tag="mm")
                        for ct in range(CT):
                            nc.tensor.matmul(
                                ps, lhsT=wsb[:, ct, mt * 128:(mt + 1) * 128],
                                rhs=src[:, ct, ch * 512:(ch + 1) * 512],
                                start=(ct == 0), stop=(ct == CT - 1))
                        if mt < 2:
                            nc.vector.tensor_copy(
                                out=kp[:, mt, ch * 512:(ch + 1) * 512], in_=ps)
                        elif mt == 2:
                            nc.vector.tensor_copy(
                                out=vpav[:, 1 + ch * hpc:1 + (ch + 1) * hpc, 1:S + 1, 1:S + 1],
                                in_=ps)
                        else:
                            nc.vector.tensor_copy(
                                out=vpbv[:, 1 + ch * hpc:1 + (ch + 1) * hpc, 1:S + 1, 1:S + 1],
                                in_=ps[0:64, :])

        # ============ Phase C/D: attention (+ interleaved MLP) =============
        # MLP work for query block nb is emitted as a closure list and
        # drained into the issue stream of attention for block nb+1.
        pG = ctx.enter_context(tc.tile_pool(name="pG", bufs=1))
        dacc = ctx.enter_context(tc.tile_pool(name="dacc", bufs=3))
        dga_t = pG.tile([128, 54, 128], BF16, tag="t_dga", name="dga_t")
        dma(out=dga_t, in_=d["dga"][:, :, :])
        dgb_t = pG.tile([64, 54, 64], BF16, tag="t_dgb", name="dgb_t")
        dma(out=dgb_t, in_=d["dgb"][:, :, :])
        dgs = {"dg2a": dga_t[:, 0:27, :], "dg1a": dga_t[:, 27:54, :],
               "dg2b": dgb_t[:, 0:27, :], "dg1b": dgb_t[:, 27:54, :]}
        DWOFFS = [(dz, dy, dx) for dz in range(3) for dy in range(3)
                  for dx in range(3)]
        ocat = pG.tile([128, 4, NQ], BF16, tag="ocat")
        # proj pad rows must be finite (pad weight rows are zero); rows
        # 32:48 / 96:112 are re-written by attention later (32-aligned access)
        nc.vector.memset(ocat[32:64, :, :], 0.0)
        nc.vector.memset(ocat[96:128, :, :], 0.0)
        pw_sb = mat_sb("pw", 4, C, pG)
        f1w_sb = mat_sb("f1w", CT, HID, pG)
        f2w_sb = mat_sb("f2w", 12, C, pG)
        zt = pG.tile([128, CT, NQ], F32, tag="zt")
        h1 = pG.tile([128, 12, 512], BF16, tag="h1")
        xm = pG.tile([128, CT, 512], BF16, tag="xm")
        ots = pG.tile([128, 4, C], F32, tag="ots")

        with tc.tile_pool(name="pS", bufs=2, space="PSUM") as pS, \
             tc.tile_pool(name="pO", bufs=1, space="PSUM") as pO, \
             tc.tile_pool(name="pM", bufs=2, space="PSUM") as pM, \
             tc.tile_pool(name="pex", bufs=3) as pex, \
             tc.tile_pool(name="prec", bufs=1) as prec:

            def dw_chunk(br, half, ch):
                """Depthwise-conv work for one 512-token chunk/half: 27 diag
                matmuls (PSUM-accum) + evict(+bias+residual) + 4 transposes +
                direct copies into the 49-augmented v layout."""
                S = 16 if br == 2 else 8
                P = S + 2
                vn = v2n if br == 2 else v1n
                vt = vp[br][0 if half == "a" else 1]
                np_ = 128 if half == "a" else 64
                dgt = dgs[f"dg{br}{half}"]
                bia = lcb[br][0 if half == "a" else 1]
                vtv = vt.rearrange("p (h w d) -> p h w d", h=P, w=P, d=P)
                hrows = 512 // (S * S)
                pd_ = pM.tile([128, 512], F32, tag="mm", name="pd_")
                for j, (dz, dy, dx) in enumerate(DWOFFS):
                    rhs = bass.AP(
                        tensor=vt.tensor,
                        offset=vt.offset + ch * hrows * P * P
                        + dz * P * P + dy * P + dx,
                        ap=[list(vt.ap[0]), [P * P, hrows], [P, S], [1, S]])
                    nc.tensor.matmul(pd_[0:np_, :], lhsT=dgt[:, j, :],
                                     rhs=rhs, start=(j == 0), stop=(j == 26))
                acc = dacc.tile([128, 512], BF16, tag="t_acc", name="acc")
                for hp in range(hrows):
                    nc.vector.scalar_tensor_tensor(
                        out=acc[0:np_, hp * S * S:(hp + 1) * S * S],
                        in0=pd_[0:np_, hp * S * S:(hp + 1) * S * S],
                        scalar=bia,
                        in1=vtv[:, 1 + ch * hrows + hp, 1:S + 1, 1:S + 1],
                        op0=AL.add, op1=AL.add)
                for sub in range(4):
                    mt = ch * 4 + sub
                    tp = pM.tile([128, 128], BF16, tag="mm", name="tp")
                    nc.tensor.transpose(tp[:, 0:np_],
                                        acc[0:np_, sub * 128:(sub + 1) * 128],
                                        ident[0:np_, 0:np_])
                    c0 = 0 if half == "a" else 128
                    for (soff, ln, dcol) in _v49_runs(c0, np_):
                        nc.vector.tensor_copy(
                            out=vn[:, mt, dcol:dcol + ln],
                            in_=tp[:, soff:soff + ln])

            def mlp_emit(nb):
                """Closure list computing proj+LN2+MLP+store for query block nb."""
                ops = []
                nsl = slice(nb * 512, (nb + 1) * 512)

                def proj_mt(mt):
                    def f():
                        ps = pM.tile([128, 512], F32, tag="mm")
                        for kt in range(4):
                            nc.tensor.matmul(ps, lhsT=pw_sb[:, kt, mt * 128:(mt + 1) * 128],
                                             rhs=ocat[:, kt, nsl],
                                             start=(kt == 0), stop=(kt == 3))
                        nc.vector.scalar_tensor_tensor(
                            out=zt[:, mt, nsl], in0=ps, scalar=pb_sb[:, mt:mt + 1],
                            in1=xct_sb[:, mt, nsl], op0=AL.add, op1=AL.add)
                    return f
                for mt in range(CT):
                    ops.append(proj_mt(mt))

                # LN2 (single 512-token chunk)
                def ln2a():
                    zbf = pG.tile([128, CT, 512], BF16, tag="zbf", name=f"zbf{nb}")
                    nc.vector.tensor_copy(out=zbf, in_=zt[:, :, nsl])
                    st = pM.tile([128, 512], F32, tag="mm", name="l2st")
                    for ct in range(CT):
                        nc.tensor.matmul(st[0:1, :], lhsT=ones_b, rhs=zbf[:, ct, :],
                                         start=(ct == 0), stop=(ct == CT - 1))
                    for ct in range(CT):
                        sqt = rows.tile([128, 512], BF16, tag="sqc")
                        nc.scalar.activation(out=sqt, in_=zbf[:, ct, :], func=AF.Square)
                        nc.tensor.matmul(st[32:33, :], lhsT=ones_b, rhs=sqt,
                                         start=(ct == 0), stop=(ct == CT - 1))
                    sts = rows.tile([33, 512], F32, tag="sts")
                    nc.vector.tensor_copy(out=sts, in_=st[0:33, :])
                    tps = pM.tile([128, 512], F32, tag="mm", name="l2tp")
                    for j in range(4):
                        nc.tensor.transpose(tps[:, 33 * j:33 * j + 33],
                                            sts[:, j * 128:(j + 1) * 128],
                                            identf[0:33, 0:33])
                    sxq = pG.tile([128, 8], F32, tag="sxq8", name=f"sxq_l2{nb}")
                    tview = bass.AP(tensor=tps.tensor, offset=tps.offset,
                                    ap=[list(tps.ap[0]), [33, 4], [32, 2]])
                    nc.vector.tensor_copy(out=sxq, in_=tview)
                    sx = sxq.rearrange("p (k two) -> p two k", two=2)
                    rm = pG.tile([128, 4], F32, tag="rm4")
                    rq = pG.tile([128, 4], F32, tag="rq4")
                    nc.vector.tensor_scalar_mul(out=rm, in0=sx[:, 0], scalar1=1.0 / C)
                    nc.vector.tensor_scalar_mul(out=rq, in0=sx[:, 1], scalar1=1.0 / C)
                    rv = pG.tile([128, 4], F32, tag="rv4")
                    nc.vector.tensor_mul(rv, rm, rm)
                    nc.vector.tensor_sub(rv, rq, rv)
                    nc.scalar.activation(out=rv, in_=rv, func=AF.Sqrt, bias=eps_t)
                    ab2 = pG.tile([128, 8], F32, tag="ab8", name=f"ab2_l2{nb}")
                    ab2v = ab2.rearrange("p (k two) -> p two k", two=2)
                    nc.vector.reciprocal(out=ab2v[:, 0], in_=rv)
                    nc.vector.scalar_tensor_tensor(out=ab2v[:, 1], in0=rm, scalar=-1.0,
                                                   in1=ab2v[:, 0], op0=AL.mult, op1=AL.mult)
                    rab = pM.tile([128, 512], F32, tag="mm", name="l2rab")
                    for j in range(4):
                        nc.tensor.transpose(rab[0:2, j * 128:(j + 1) * 128],
                                            ab2[:, 2 * j:2 * j + 2], identf)
                    abrow = rows.tile([2, 512], BF16, tag="abrow", name=f"abrow_l2{nb}")
                    nc.vector.tensor_copy(out=abrow, in_=rab[0:2, :])
                    # broadcast (two sequential single-bank PSUM tiles)
                    abpa = pM.tile([128, 512], F32, tag="mm", name="l2aba")
                    nc.tensor.matmul(abpa, lhsT=sel2[:, 0, :],
                                     rhs=abrow[0:2, :], start=True, stop=True)
                    abpb = pM.tile([128, 512], F32, tag="mm", name="l2abb")
                    nc.tensor.matmul(abpb, lhsT=sel2[:, 1, :],
                                     rhs=abrow[0:2, :], start=True, stop=True)
                    tmp3 = pG.tile([128, 512], F32, tag="tmp3")
                    for ct in range(CT):
                        nc.vector.tensor_mul(tmp3, zt[:, ct, nsl], abpa)
                        nc.vector.tensor_add(tmp3, tmp3, abpb)
                        nc.scalar.activation(out=xm[:, ct, :], in_=tmp3, func=AF.Identity,
                                             bias=ln2b_sb[:, ct:ct + 1],
                                             scale=ln2w_sb[:, ct:ct + 1])
                ops.append(ln2a)

                def fc1_mt(mt):
                    def f():
                        ps = pM.tile([128, 512], F32, tag="mm")
                        for ct in range(CT):
                            nc.tensor.matmul(ps, lhsT=f1w_sb[:, ct, mt * 128:(mt + 1) * 128],
                                             rhs=xm[:, ct, :],
                                             start=(ct == 0), stop=(ct == CT - 1))
                        nc.scalar.activation(out=h1[:, mt, :], in_=ps, func=AF.Gelu,
                                             bias=f1b_sb[:, mt:mt + 1])
                    return f
                for mt in range(12):
                    ops.append(fc1_mt(mt))

                def fc2_mt(mt):
                    def f():
                        ps = pM.tile([128, 512], F32, tag="mm")
                        for kt in range(12):
                            nc.tensor.matmul(ps, lhsT=f2w_sb[:, kt, mt * 128:(mt + 1) * 128],
                                             rhs=h1[:, kt, :],
                                             start=(kt == 0), stop=(kt == 11))
                        nc.vector.scalar_tensor_tensor(
                            out=zt[:, mt, nsl], in0=ps, scalar=f2b_sb[:, mt:mt + 1],
                            in1=zt[:, mt, nsl], op0=AL.add, op1=AL.add)
                    return f
                for mt in range(CT):
                    ops.append(fc2_mt(mt))

                def store_nt(nt):
                    def f():
                        for ct in range(CT):
                            tp2 = pM.tile([128, 512], F32, tag="mm", name="otp")
                            nc.tensor.transpose(
                                tp2[:, 0:128],
                                zt[:, ct, nb * 512 + nt * 128:nb * 512 + (nt + 1) * 128],
                                identf)
                            nc.vector.tensor_copy(out=ots[:, nt, ct * 128:(ct + 1) * 128],
                                                  in_=tp2[:, 0:128])
                    return f
                for nt in range(4):
                    ops.append(store_nt(nt))

                def store_dma():
                    dma(out=out_d[nb * 512:(nb + 1) * 512, :].rearrange(
                        "(nt p) c -> p nt c", p=128),
                        in_=ots)
                ops.append(store_dma)
                return ops

            def attention(nb, filler, dw_groups=None):
                """Attention for query block nb; drains filler closures into
                the issue stream (~1 per pipeline unit). dw_groups maps
                (br, group) -> closures that must run before AV of mt in
                [4*group, 4*group+4), hiding the depthwise conv under the
                exp stream."""
                def drain(k):
                    for _ in range(k):
                        if filler:
                            filler.pop(0)()
                for (br, kp, qp, vn, nmt) in ((2, k2p, q2p, v2n, N // 128),
                                              (1, k1p, q1p, v1n, N1 // 128)):
                    done_g = set()

                    def emit_dw(ci):
                        if dw_groups and ci not in done_g:
                            done_g.add(ci)
                            for fn in dw_groups.get((br, ci), ()):
                                fn()
                    emit_dw(0)
                    oa = pO.tile([128, 512], F32, tag="oa")
                    ob = pO.tile([128, 512], F32, tag="ob")
                    units = [(mt, g) for mt in range(nmt) for g in range(2)]

                    S_t = {}
                    ex_t = {}

                    def emit_S(u):
                        mt, g = units[u]
                        St = pS.tile([128, 1024], F32, tag="S", name=f"S{nb}{br}")
                        for r in range(2):
                            nc.tensor.matmul(
                                St[:, r * 512:(r + 1) * 512],
                                lhsT=kp[64 * r:64 * r + 64, g, mt * 128:(mt + 1) * 128],
                                rhs=qp[64 * r:64 * r + 64, g, nb * 512:(nb + 1) * 512],
                                start=True, stop=True, tile_position=(64 * r, 0))
                        ex = pex.tile([128, 1024], BF16, tag="ex")
                        nc.scalar.activation(out=ex, in_=St, func=AF.Exp)
                        S_t[u] = St
                        ex_t[u] = ex

                    def emit_AV(u):
                        mt, g = units[u]
                        ot = oa if g == 0 else ob
                        ex = ex_t.pop(u)
                        S_t.pop(u)
                        for r in range(2):
                            h = 2 * g + r
                            nc.tensor.matmul(ot[64 * r:64 * r + 49, :],
                                             lhsT=vn[:, mt, h * 49:(h + 1) * 49],
                                             rhs=ex[:, r * 512:(r + 1) * 512],
                                             start=(mt == 0), stop=(mt == nmt - 1),
                                             tile_position=(0, 64 * r))

                    emit_S(0)
                    for u in range(len(units)):
                        mt, g = units[u]
                        if g == 0 and mt % 4 == 0:
                            emit_dw(mt // 4 + 1)
                        if u + 1 < len(units):
                            emit_S(u + 1)
                        emit_AV(u)
                        drain(1)

                    # reciprocal straight from the PSUM blocks holding the
                    # denominator rows (48/112 -> local row 16). The broadcast
                    # matmuls contract only K=17 rows (0..16) so no stale PSUM
                    # rows (49-63/113-127 of oa/ob, never written by AV) are
                    # read: stale Inf/NaN times a zero selector weight would
                    # poison the output.
                    rcs = []
                    for blk, (srcp, r0) in enumerate(((oa, 32), (oa, 96),
                                                      (ob, 32), (ob, 96))):
                        rct = prec.tile([32, 512], F32, tag=f"rc{blk}")
                        nc.vector.reciprocal(out=rct, in_=srcp[r0:r0 + 32, :])
                        rcs.append(rct)
                    sel17 = sel64t[0:17, 0:48]
                    tbase = 0 if br == 1 else 2
                    for pi, srcp in enumerate((oa, ob)):
                        rc = pS.tile([128, 1024], F32, tag="S", name=f"rc{nb}{br}{pi}")
                        nc.tensor.matmul(rc[0:48, 0:512], lhsT=sel17,
                                         rhs=rcs[2 * pi][0:17, :],
                                         start=True, stop=True)
                        nc.tensor.matmul(rc[64:112, 0:512], lhsT=sel17,
                                         rhs=rcs[2 * pi + 1][0:17, :],
                                         start=True, stop=True,
                                         tile_position=(0, 64))
                        recb = prec.tile([128, 512], F32, tag=f"recb{pi}")
                        nc.vector.tensor_copy(out=recb[0:112, :], in_=rc[0:112, 0:512])
                        tt = tbase + pi
                        for r in range(2):
                            nc.vector.tensor_mul(
                                ocat[64 * r:64 * r + 48, tt, nb * 512:(nb + 1) * 512],
                                srcp[64 * r:64 * r + 48, :], recb[64 * r:64 * r + 48, :])
                    drain(4)

            if "noattn" not in ABL:
                def mk_dw(br, half, ch):
                    return lambda: dw_chunk(br, half, ch)
                dwg = {}
                if "nodw" not in ABL:
                    for ch in range(8):
                        dwg[(2, ch)] = [mk_dw(2, "a", ch), mk_dw(2, "b", ch)]
                    dwg[(1, 0)] = [mk_dw(1, "a", 0), mk_dw(1, "b", 0)]
                attention(0, [], dwg)
                fill = mlp_emit(0)
                attention(1, fill)
                for f in fill:
                    f()
                for f in mlp_emit(1):
                    f()


_PROG = None


def _get_program():
    global _PROG
    if _PROG is None:
        _PROG = build_program()
    return _PROG


def _diag(w):
    """[n, 27] weights -> [n, 27, n] per-offset diagonal matrices (bf16)."""
    n = w.shape[0]
    out = np.zeros((n, 27, n), BF)
    idx = np.arange(n)
    for j in range(27):
        out[idx, j, idx] = w[:, j].astype(BF)
    return out


def _pad_heads_out(w384, bias=None):
    """[C_in, 384] head-major (8x48) -> [C_in, 512] with 64-row head slots."""
    cin = w384.shape[0]
    out = np.zeros((cin, 512), np.float32)
    bout = np.zeros(512, np.float32)
    for h in range(8):
        tt, e = h // 2, h % 2
        out[:, 128 * tt + 64 * e:128 * tt + 64 * e + 48] = w384[:, h * 48:(h + 1) * 48]
        if bias is not None:
            bout[128 * tt + 64 * e:128 * tt + 64 * e + 48] = bias[h * 48:(h + 1) * 48]
    return out, bout


def _pad_kv(wT):
    """kv weight [C_in, 384] (k 4x48 | v 192) -> [C_in, 512]:
    tiles 0,1 = k head slots, tile 2 = v 0:128, tile 3 = v 128:192 + pad."""
    cin = wT.shape[0]
    out = np.zeros((cin, 512), np.float32)
    for h in range(4):
        tt, e = h // 2, h % 2
        out[:, 128 * tt + 64 * e:128 * tt + 64 * e + 48] = wT[:, h * 48:(h + 1) * 48]
    out[:, 256:384] = wT[:, 192:320]
    out[:, 384:448] = wT[:, 320:384]
    return out


def kernel(x, ln1_w, ln1_b, q_w, sr1_w, sr1_b, n1_w, n1_b, sr2_w, sr2_b,
           n2_w, n2_b, kv1_w, kv2_w, lc1_w, lc1_b, lc2_w, lc2_b,
           proj_w, proj_b, ln2_w, ln2_b, fc1_w, fc1_b, fc2_w, fc2_b,
           H, W, D):
    f = lambda a: np.asarray(a, np.float32)
    x = f(x)
    ln1_w, ln1_b = f(ln1_w), f(ln1_b)
    qs = HD ** -0.5
    lc1 = f(lc1_w).reshape(C2, 27)
    lc2 = f(lc2_w).reshape(C2, 27)

    qwp, qbp = _pad_heads_out((f(q_w) * ln1_w[None, :]).T * qs,
                              f(q_w) @ ln1_b * qs)
    # proj: padded input rows (512) matching ocat head-slot layout
    pwp = np.zeros((512, C), np.float32)
    pT = f(proj_w).T  # [384 in, 384 out]
    for g in range(8):
        tt, e = g // 2, g % 2
        pwp[128 * tt + 64 * e:128 * tt + 64 * e + 48, :] = pT[g * 48:(g + 1) * 48, :]

    # pack all bias/affine vectors into one [128, 50] f32 tensor
    def pack_pt(v, ncol):
        return f(v).reshape(ncol, 128).T

    vecs = np.zeros((128, 50), np.float32)
    vecs[:, 0:4] = pack_pt(qbp, 4)
    vecs[:, 4:7] = pack_pt(f(sr2_b) + f(sr2_w)[:, :, 0, 0, 0] @ ln1_b, 3)
    vecs[:, 7:10] = pack_pt(f(sr1_b) + np.einsum("ocijk,c->o", f(sr1_w), ln1_b), 3)
    vecs[:, 10:13] = pack_pt(n1_w, 3)
    vecs[:, 13:16] = pack_pt(n1_b, 3)
    vecs[:, 16:19] = pack_pt(n2_w, 3)
    vecs[:, 19:22] = pack_pt(n2_b, 3)
    vecs[:, 22:25] = pack_pt(proj_b, 3)
    vecs[:, 25:28] = pack_pt(ln2_w, 3)
    vecs[:, 28:31] = pack_pt(ln2_b, 3)
    vecs[:, 31:34] = pack_pt(fc2_b, 3)
    vecs[:, 34:46] = pack_pt(fc1_b, 12)
    vecs[:, 46] = f(lc1_b)[0:128]
    vecs[0:64, 47] = f(lc1_b)[128:192]
    vecs[:, 48] = f(lc2_b)[0:128]
    vecs[0:64, 49] = f(lc2_b)[128:192]

    wcat = np.concatenate([
        qwp,
        _pad_kv(f(kv2_w).T),
        _pad_kv(f(kv1_w).T),
        (f(sr2_w)[:, :, 0, 0, 0] * ln1_w[None, :]).T,
    ], axis=1)

    sel2 = np.zeros((2, 2, 128), np.float32)
    sel2[0, 0, :] = 1.0
    sel2[1, 1, :] = 1.0
    sel64 = np.zeros((64, 2, 48), np.float32)
    sel64[16, 0, :] = 1.0
    sel64[48, 1, :] = 1.0

    wm = {
        "vecs": vecs,
        "sel2": sel2.reshape(2, 256).astype(BF),
        "sel64": sel64.reshape(64, 96),
        "wcat": np.ascontiguousarray(wcat).astype(BF),
        "s1w": np.ascontiguousarray(
            (f(sr1_w) * ln1_w[None, :, None, None, None])
            .transpose(2, 3, 4, 1, 0).reshape(8, C, C)).astype(BF),
        "dga": np.concatenate([_diag(lc2[0:128]), _diag(lc1[0:128])], axis=1),
        "dgb": np.concatenate([_diag(lc2[128:192]), _diag(lc1[128:192])], axis=1),
        "pw": np.ascontiguousarray(pwp).astype(BF),
        "f1w": np.ascontiguousarray(f(fc1_w).T).astype(BF),
        "f2w": np.ascontiguousarray(f(fc2_w).T).astype(BF),
    }

    in_maps = []
    for core in range(8):
        b, qc = core // 4, core % 4
        xtb = x[b].T
        m = dict(wm)
        m["xt"] = np.ascontiguousarray(xtb).astype(BF)
        m["xct"] = np.ascontiguousarray(xtb[:, qc * NQ:(qc + 1) * NQ]).astype(np.float32)
        in_maps.append(m)

    nc = _get_program()
    res = run_bass_kernel_spmd(nc, in_maps, list(range(8)))

    out = np.empty((B, N, C), np.float32)
    for core in range(8):
        b, qc = core // 4, core % 4
        out[b, qc * NQ:(qc + 1) * NQ, :] = res.results[core]["out"]
    return out
